# revision 10
# baseline (speedup 1.0000x reference)
"""Trainium2 Bass kernel for multi-head attention (B=4, H=8, L=2048, dim=512).

Sharding: 8 cores = 4 batches x 2 sequence halves. Each core computes the
full attention output for one batch's 1024-query half (all 8 heads), using
K/V over the full 2048-key sequence; the output projection contracts the
full hidden dim locally, so no cross-core communication is needed.

v3 changes vs baseline:
  - bf16 attention operands (fp16 moving operands stream at ~half rate).
  - The softmax 1/sqrt(d) scale rides the exp's free scale parameter.
  - Prologue DMAs split across the two hardware DGE queues (sync +
    scalar) so the input loads run in parallel.
  - Early-start attention: only Q/K for head pair 0 (keys 0:1024) project
    in the prologue; remaining projection groups interleave one-per-kt
    into the attention loop.
  - Shorter end-of-kernel critical chain for the last pair's softmax
    denominators (eager running folds).
"""
import numpy as np

import concourse.bass as bass
import concourse.tile as tile
from concourse import bacc, mybir
from concourse.bass_utils import run_bass_kernel_spmd

F16 = mybir.dt.bfloat16
F32 = mybir.dt.float32
P = 128
D = 512          # model dim
L = 2048         # full sequence (keys)
QL = 1024        # per-core query length
H = 8            # heads
C = 64           # head dim
HID = 512        # H * C
DC = D // P      # 4 contraction chunks
KT = L // P      # 16 key tiles
N = 512          # matmul free-dim chunk
QC = QL // N     # 2 query chunks
LC = L // N      # 4 key free-dim chunks
SCALE = C ** -0.5
EXP = mybir.ActivationFunctionType.Exp
I32 = mybir.dt.int32
# Schraudolph fast-exp: exp(SCALE*s) ~= bitcast_f32(int32(SEXP_A*s + SEXP_B))
SEXP_A = SCALE * (2 ** 23) / float(np.log(2.0))
SEXP_B = float(127 * 2 ** 23 - 490000)
SEXP_KTS = ()  # kts whose A-head exp runs on VectorE instead of ScalarE


def emit(nc, tc, x, wq, wk, wv, wo, bias, out):
    import contextlib
    ctx = contextlib.ExitStack()
    with ctx:
        # ---- pools -----------------------------------------------------
        consts = ctx.enter_context(tc.tile_pool(name="consts", bufs=1))
        qkv = ctx.enter_context(tc.tile_pool(name="qkv", bufs=1))
        ph1 = ctx.enter_context(tc.tile_pool(name="ph1", bufs=1))
        atp = ctx.enter_context(tc.tile_pool(name="atp", bufs=10))
        t8p = ctx.enter_context(tc.tile_pool(name="t8p", bufs=2))
        t4p = ctx.enter_context(tc.tile_pool(name="t4p", bufs=2))
        t2p = ctx.enter_context(tc.tile_pool(name="t2p", bufs=2))
        t1p = ctx.enter_context(tc.tile_pool(name="t1p", bufs=2))
        rbp = ctx.enter_context(tc.tile_pool(name="rbp", bufs=1))
        otup = ctx.enter_context(tc.tile_pool(name="otup", bufs=2))
        outp = ctx.enter_context(tc.tile_pool(name="outp", bufs=2))
        sip = ctx.enter_context(tc.tile_pool(name="sip", bufs=2))
        # PSUM: ps 2x2 banks + po 1x2 + pss 2x1 = 8 banks.
        pp_s = ctx.enter_context(tc.tile_pool(name="pps", bufs=2, space="PSUM"))
        pp_o = ctx.enter_context(tc.tile_pool(name="ppo", bufs=1, space="PSUM"))
        pp_sum = ctx.enter_context(tc.tile_pool(name="ppsum", bufs=2, space="PSUM"))

        # ---- persistent SBUF ------------------------------------------
        wo_sb = consts.tile([P, DC, HID], F16)
        bias_sb = consts.tile([P, DC], F32)
        ones_sb = consts.tile([P, C], F16)
        nc.vector.memset(ones_sb[:], 1.0)

        q_sb = qkv.tile([P, DC, QL], F16)
        k_sb = qkv.tile([P, DC, L], F16)
        vt_sb = qkv.tile([P, KT, HID], F16)
        ot_sb = qkv.tile([P, DC, QL], F16)

        # ---- DMA loads -------------------------------------------------
        x_sb = ph1.tile([P, DC, L], F16)
        wq_sb = ph1.tile([P, DC, HID], F16)
        wk_sb = ph1.tile([P, DC, HID], F16)
        wv_sb = ph1.tile([P, DC, HID], F16)
        xr = x.rearrange("(a p) n -> p a n", p=P)
        # query-half of x + wq/wk first so the pair-0 Q/K projections and
        # the attention loop start ASAP; wv next for the interleaved V^T.
        nc.scalar.dma_start(out=wq_sb[:], in_=wq.rearrange("(a p) n -> p a n", p=P))
        nc.sync.dma_start(out=x_sb[:, :, 0:N], in_=xr[:, :, 0:N])
        nc.scalar.dma_start(out=x_sb[:, :, N:QL], in_=xr[:, :, N:QL])
        nc.sync.dma_start(out=wk_sb[:], in_=wk.rearrange("(a p) n -> p a n", p=P))
        nc.scalar.dma_start(out=wv_sb[:], in_=wv.rearrange("(a p) n -> p a n", p=P))
        nc.sync.dma_start(out=x_sb[:, :, QL:L], in_=xr[:, :, QL:L])
        nc.scalar.dma_start(out=wo_sb[:], in_=wo.rearrange("(a p) n -> p a n", p=P))
        nc.sync.dma_start(out=bias_sb[:], in_=bias)

        def q_group(m, qc):
            # Q chunk via fp8 DoubleRow: two dc-chunks contracted per pass
            ps = pp_sum.tile([P, N], F32, tag="pss", name=f"qg{m}_{qc}")
            for dc in range(DC):
                nc.tensor.matmul(
                    ps[:],
                    lhsT=wq_sb[:, dc, m * P:(m + 1) * P],
                    rhs=x_sb[:, dc, qc * N:(qc + 1) * N],
                    start=(dc == 0), stop=(dc == DC - 1),
                )
            nc.vector.tensor_copy(q_sb[:, m, qc * N:(qc + 1) * N], ps[:])

        def k_group(m, lc):
            ps = pp_sum.tile([P, N], F32, tag="pss", name=f"kg{m}_{lc}")
            for dc in range(DC):
                nc.tensor.matmul(
                    ps[:],
                    lhsT=wk_sb[:, dc, m * P:(m + 1) * P],
                    rhs=x_sb[:, dc, lc * N:(lc + 1) * N],
                    start=(dc == 0), stop=(dc == DC - 1),
                )
            nc.vector.tensor_copy(k_sb[:, m, lc * N:(lc + 1) * N], ps[:])

        def vt_proj(kt):
            # V^T: [k, hc] (x stationary); interleaved into pair-0's kt loop.
            ps = pp_sum.tile([P, N], F32, tag="pss", name=f"vtp{kt}")
            for dc in range(DC):
                nc.tensor.matmul(
                    ps[:],
                    lhsT=x_sb[:, dc, kt * P:(kt + 1) * P],
                    rhs=wv_sb[:, dc, :],
                    start=(dc == 0), stop=(dc == DC - 1),
                )
            nc.vector.tensor_copy(vt_sb[:, kt, :], ps[:])

        # Deferred projection queue, emitted into the attention loop.
        # Pair 0 (odd kts, 8 slots): K[0] tail + Q[1]/K[1] (needed by the
        # pair-1 scores). Pairs 1-2 (even kts) pick up the rest.
        proj_queue = [("k", 0, 2), ("k", 0, 3), ("q", 1, 0), ("q", 1, 1),
                      ("k", 1, 0), ("k", 1, 1), ("k", 1, 2), ("k", 1, 3)]
        for m in range(2, DC):
            proj_queue.append(("q", m, 0))
            proj_queue.append(("q", m, 1))
            for lc in range(LC):
                proj_queue.append(("k", m, lc))

        def emit_proj_group():
            if not proj_queue:
                return
            kind, m, i = proj_queue.pop(0)
            if kind == "q":
                q_group(m, i)
            else:
                k_group(m, i)

        # ---- prologue: pair-0 Q + K (first half of keys) ---------------
        q_group(0, 0)
        q_group(0, 1)
        k_group(0, 0)
        k_group(0, 1)

        def finish_tail(st):
            # denominators part 2: partition-sum via the all-ones [128, 64]
            # stationary matmul, reciprocal, then the deferred
            # normalization multiply (runs early in the NEXT pair).
            m, otu, t1s = st
            rbr = rbp.tile([P, QL], F32, tag="rbr", name=f"rbr{m}")
            for qc in range(QC):
                rb_ps = pp_sum.tile([P, N], F32, tag="pss", name=f"rb{m}_{qc}")
                for half in range(2):
                    nc.tensor.matmul(
                        rb_ps[half * C:(half + 1) * C, :],
                        lhsT=ones_sb[:],
                        rhs=t1s[half][:, qc * N:(qc + 1) * N],
                        start=True, stop=True,
                        tile_position=(0, half * C), skip_group_check=True,
                    )
                nc.vector.reciprocal_approx_fast(
                    out=rbr[:, qc * N:(qc + 1) * N], in_=rb_ps[:]
                )
            nc.vector.tensor_mul(ot_sb[:, m, :], otu[:], rbr[:])

        pending = None
        # ---- attention: one head pair (2m, 2m+1) at a time -------------
        for m in range(DC):
            po = pp_o.tile([P, QL], F32, tag="po", name=f"po{m}")
            t8_A = t8p.tile([P, KT // 2, QL], F16, tag="t8", name=f"t8a{m}")
            t8_B = t8p.tile([P, KT // 2, QL], F16, tag="t8", name=f"t8b{m}")
            t4_A = t4p.tile([P, KT // 4, QL], F16, tag="t4", name=f"t4a{m}")
            t4_B = t4p.tile([P, KT // 4, QL], F16, tag="t4", name=f"t4b{m}")
            last = m == DC - 1
            if last:
                # eager running-fold buffers for a short final chain
                t2_A3 = t2p.tile([P, 2, QL], F16, tag="t2", name="t2a3")
                t2_B3 = t2p.tile([P, 2, QL], F16, tag="t2", name="t2b3")

            def emit_l1(kt, at_prev, at_A, at_B):
                j = kt // 2
                nc.vector.tensor_add(t8_A[:, j, :], at_prev[0][:], at_A[:])
                nc.vector.tensor_add(t8_B[:, j, :], at_prev[1][:], at_B[:])
                if kt % 4 == 3:
                    i = kt // 4
                    nc.vector.tensor_add(
                        t4_A[:, i, :], t8_A[:, 2 * i, :], t8_A[:, 2 * i + 1, :]
                    )
                    nc.vector.tensor_add(
                        t4_B[:, i, :], t8_B[:, 2 * i, :], t8_B[:, 2 * i + 1, :]
                    )
                    if last and i == 1:
                        # t2run = t4[0] + t4[1]
                        nc.vector.tensor_add(
                            t2_A3[:, 0, :], t4_A[:, 0, :], t4_A[:, 1, :]
                        )
                        nc.vector.tensor_add(
                            t2_B3[:, 0, :], t4_B[:, 0, :], t4_B[:, 1, :]
                        )
                    if last and i == 2:
                        # t3run = t2run + t4[2]
                        nc.vector.tensor_add(
                            t2_A3[:, 1, :], t2_A3[:, 0, :], t4_A[:, 2, :]
                        )
                        nc.vector.tensor_add(
                            t2_B3[:, 1, :], t2_B3[:, 0, :], t4_B[:, 2, :]
                        )
                if last and kt == 13:
                    # t3b = t3run + t8[6]  (so the post-loop chain is only
                    # two adds: u = at14+at15; t1 = t3b + u)
                    nc.vector.tensor_add(
                        t2_A3[:, 0, :], t2_A3[:, 1, :], t8_A[:, 6, :]
                    )
                    nc.vector.tensor_add(
                        t2_B3[:, 0, :], t2_B3[:, 1, :], t8_B[:, 6, :]
                    )

            at_prev = [None, None]  # odd-kt pairing for the L1 adds
            attnv0_pending = None
            for kt in range(KT):
                if m == 0:
                    vt_proj(kt)
                    if kt % 2 == 1:
                        emit_proj_group()
                elif kt % 2 == 0:
                    emit_proj_group()
                if pending is not None and kt == 2:
                    finish_tail(pending)
                    pending = None
                ps_A = pp_s.tile([P, QL], F32, tag="ps")
                ps_B = pp_s.tile([P, QL], F32, tag="ps")
                at_A = atp.tile([P, QL], F16, tag="at")
                at_B = atp.tile([P, QL], F16, tag="at")
                for qc in range(QC):
                    nc.tensor.matmul(
                        ps_A[:, qc * N:(qc + 1) * N],
                        lhsT=k_sb[0:C, m, kt * P:(kt + 1) * P],
                        rhs=q_sb[0:C, m, qc * N:(qc + 1) * N],
                        start=True, stop=True, tile_position=(0, 0),
                    )
                for qc in range(QC):
                    nc.tensor.matmul(
                        ps_B[:, qc * N:(qc + 1) * N],
                        lhsT=k_sb[C:P, m, kt * P:(kt + 1) * P],
                        rhs=q_sb[C:P, m, qc * N:(qc + 1) * N],
                        start=True, stop=True, tile_position=(C, 0),
                    )
                if kt in SEXP_KTS:
                    si = sip.tile([P, QL], I32, tag="si")
                    nc.vector.tensor_scalar(
                        si[:], ps_A[:], SEXP_A, SEXP_B,
                        op0=mybir.AluOpType.mult, op1=mybir.AluOpType.add,
                    )
                    nc.vector.tensor_copy(at_A[:], si[:].bitcast(F32))
                else:
                    nc.scalar.activation(at_A[:], ps_A[:], EXP, scale=SCALE)
                nc.scalar.activation(at_B[:], ps_B[:], EXP, scale=SCALE)

                def attnv(kt, at_A, at_B):
                    # attn @ V: col-packed pair, accumulate over kt.
                    for qc in range(QC):
                        nc.tensor.matmul(
                            po[0:C, qc * N:(qc + 1) * N],
                            lhsT=vt_sb[:, kt, (2 * m) * C:(2 * m + 1) * C],
                            rhs=at_A[:, qc * N:(qc + 1) * N],
                            start=(kt == 0), stop=(kt == KT - 1),
                            tile_position=(0, 0), skip_group_check=True,
                        )
                        nc.tensor.matmul(
                            po[C:P, qc * N:(qc + 1) * N],
                            lhsT=vt_sb[:, kt, (2 * m + 1) * C:(2 * m + 2) * C],
                            rhs=at_B[:, qc * N:(qc + 1) * N],
                            start=(kt == 0), stop=(kt == KT - 1),
                            tile_position=(0, C), skip_group_check=True,
                        )

                # Defer attnV(0) past scores(1): the first attnV of a pair
                # waits on the previous pair's po drain; emitting it after
                # the next scores keeps the in-order PE queue moving.
                if kt == 0:
                    attnv0_pending = (kt, at_A, at_B)
                else:
                    if attnv0_pending is not None:
                        attnv(*attnv0_pending)
                        attnv0_pending = None
                    attnv(kt, at_A, at_B)
                if kt % 2 == 0:
                    at_prev = [at_A, at_B]
                elif kt < KT - 1:
                    emit_l1(kt, at_prev, at_A, at_B)
                else:
                    last_l1 = (kt, at_prev, at_A, at_B)

            otu = otup.tile([P, QL], F16, tag="otu", name=f"otu{m}")
            nc.vector.tensor_copy(otu[:], po[:])

            # denominators part 1: fold to t1 on DVE
            t1s = []
            if last:
                kt, at_prev, at_A, at_B = last_l1
                for at_p, at_k, t2_t in ((at_prev[0], at_A, t2_A3),
                                         (at_prev[1], at_B, t2_B3)):
                    u = t1p.tile([P, QL], F16, tag="t1")
                    nc.vector.tensor_add(u[:], at_p[:], at_k[:])
                    t1 = t1p.tile([P, QL], F16, tag="t1")
                    nc.vector.tensor_add(t1[:], t2_t[:, 0, :], u[:])
                    t1s.append(t1)
            else:
                emit_l1(*last_l1)
                for t4_t in (t4_A, t4_B):
                    r4 = t4_t[:].rearrange("p (a b) q -> p a b q", a=2)
                    t2 = t2p.tile([P, KT // 8, QL], F16, tag="t2")
                    nc.vector.tensor_add(t2[:], r4[:, 0], r4[:, 1])
                    t1 = t1p.tile([P, QL], F16, tag="t1")
                    nc.vector.tensor_add(t1[:], t2[:, 0, :], t2[:, 1, :])
                    t1s.append(t1)
            pending = (m, otu, t1s)

        finish_tail(pending)
        assert not proj_queue, f"{len(proj_queue)} proj groups left"

        # ---- output projection + bias (fp8 DoubleRow) ------------------
        for mo in range(DC):
            for qc in range(QC):
                ps = pp_s.tile([P, QL], F32, tag="ps", name=f"po3_{mo}_{qc}")
                for mh in range(DC):
                    nc.tensor.matmul(
                        ps[:, 0:N],
                        lhsT=wo_sb[:, mh, mo * P:(mo + 1) * P],
                        rhs=ot_sb[:, mh, qc * N:(qc + 1) * N],
                        start=(mh == 0), stop=(mh == DC - 1),
                    )
                ob = outp.tile([P, N], F32, tag="ob")
                nc.vector.tensor_scalar_add(ob[:], ps[:, 0:N], bias_sb[:, mo:mo + 1])
                nc.sync.dma_start(
                    out=out[mo * P:(mo + 1) * P, qc * N:(qc + 1) * N], in_=ob[:]
                )


def build():
    nc = bacc.Bacc("TRN2", target_bir_lowering=False, debug=False)
    x = nc.dram_tensor("x", [D, L], F16, kind="ExternalInput").ap()
    wq = nc.dram_tensor("wq", [D, HID], F16, kind="ExternalInput").ap()
    wk = nc.dram_tensor("wk", [D, HID], F16, kind="ExternalInput").ap()
    wv = nc.dram_tensor("wv", [D, HID], F16, kind="ExternalInput").ap()
    wo = nc.dram_tensor("wo", [HID, D], F16, kind="ExternalInput").ap()
    bias = nc.dram_tensor("bias", [P, DC], F32, kind="ExternalInput").ap()
    out = nc.dram_tensor("out", [D, QL], F32, kind="ExternalOutput").ap()
    with tile.TileContext(nc) as tc:
        emit(nc, tc, x, wq, wk, wv, wo, bias, out)
    nc.compile()
    return nc


_NC_CACHE = None


def _get_nc():
    global _NC_CACHE
    if _NC_CACHE is None:
        _NC_CACHE = build()
    return _NC_CACHE


def make_in_maps(x, w_qkv, w_out, b_out):
    """Host-side sharding: returns the 8 per-core input dicts."""
    from ml_dtypes import bfloat16
    f8 = bfloat16
    # SCALE is applied inside the kernel via the exp's scale parameter
    # (folding it into fp8 wq would land in subnormal range).
    wq_t = np.ascontiguousarray(w_qkv[0:HID].T).astype(f8)
    wk_t = np.ascontiguousarray(w_qkv[HID:2 * HID].T).astype(f8)
    wv_t = np.ascontiguousarray(w_qkv[2 * HID:3 * HID].T).astype(f8)
    wo_t = np.ascontiguousarray(w_out.T).astype(f8)
    bias = np.ascontiguousarray(b_out.reshape(DC, P).T).astype(np.float32)
    in_maps = []
    for core in range(8):
        b, halfq = core // 2, core % 2
        # rotate so this core's query half sits at columns 0:QL; key order
        # is irrelevant (softmax sums over all keys).
        x_rot = np.roll(x[b], -halfq * QL, axis=1).astype(f8)
        in_maps.append({
            "x": np.ascontiguousarray(x_rot),
            "wq": wq_t, "wk": wk_t, "wv": wv_t, "wo": wo_t,
            "bias": bias,
        })
    return in_maps


def assemble(results):
    out = np.zeros((4, D, L), np.float32)
    for core in range(8):
        b, halfq = core // 2, core % 2
        out[b][:, halfq * QL:(halfq + 1) * QL] = results[core]["out"]
    return out


def kernel(x, w_qkv, w_out, b_out):
    x = np.asarray(x, np.float32)
    w_qkv = np.asarray(w_qkv, np.float32)
    w_out = np.asarray(w_out, np.float32)
    b_out = np.asarray(b_out, np.float32)
    nc = _get_nc()
    in_maps = make_in_maps(x, w_qkv, w_out, b_out)
    res = run_bass_kernel_spmd(nc, in_maps, core_ids=list(range(8)))
    return assemble(res.results)


# revision 13
# speedup vs baseline: 1.0101x; 1.0101x over previous
"""Trainium2 Bass kernel for multi-head attention (B=4, H=8, L=2048, dim=512).

Sharding: 8 cores = 4 batches x 2 sequence halves. Each core computes the
full attention output for one batch's 1024-query half (all 8 heads), using
K/V over the full 2048-key sequence; the output projection contracts the
full hidden dim locally, so no cross-core communication is needed.

v3 changes vs baseline:
  - bf16 attention operands (fp16 moving operands stream at ~half rate).
  - The softmax 1/sqrt(d) scale rides the exp's free scale parameter.
  - Prologue DMAs split across the two hardware DGE queues (sync +
    scalar) so the input loads run in parallel.
  - Early-start attention: the prologue projects only pair-0 Q/K plus
    ten V^T tiles (filling the otherwise idle DMA window); the remaining
    projection groups interleave one-per-kt into the attention loop.
  - Shorter end-of-kernel critical chain for the last pair's softmax
    denominators (eager running folds).
"""
import numpy as np

import concourse.bass as bass
import concourse.tile as tile
from concourse import bacc, mybir
from concourse.bass_utils import run_bass_kernel_spmd

F16 = mybir.dt.bfloat16
F32 = mybir.dt.float32
P = 128
D = 512          # model dim
L = 2048         # full sequence (keys)
QL = 1024        # per-core query length
H = 8            # heads
C = 64           # head dim
HID = 512        # H * C
DC = D // P      # 4 contraction chunks
KT = L // P      # 16 key tiles
N = 512          # matmul free-dim chunk
QC = QL // N     # 2 query chunks
LC = L // N      # 4 key free-dim chunks
SCALE = C ** -0.5
EXP = mybir.ActivationFunctionType.Exp
I32 = mybir.dt.int32
# Schraudolph fast-exp: exp(SCALE*s) ~= bitcast_f32(int32(SEXP_A*s + SEXP_B))
SEXP_A = SCALE * (2 ** 23) / float(np.log(2.0))
SEXP_B = float(127 * 2 ** 23 - 490000)
SEXP_KTS = ()  # kts whose A-head exp runs on VectorE instead of ScalarE


def emit(nc, tc, x, wq, wk, wv, wo, bias, out):
    import contextlib
    ctx = contextlib.ExitStack()
    with ctx:
        # ---- pools -----------------------------------------------------
        consts = ctx.enter_context(tc.tile_pool(name="consts", bufs=1))
        qkv = ctx.enter_context(tc.tile_pool(name="qkv", bufs=1))
        ph1 = ctx.enter_context(tc.tile_pool(name="ph1", bufs=1))
        atp = ctx.enter_context(tc.tile_pool(name="atp", bufs=10))
        t8p = ctx.enter_context(tc.tile_pool(name="t8p", bufs=2))
        t4p = ctx.enter_context(tc.tile_pool(name="t4p", bufs=2))
        t2p = ctx.enter_context(tc.tile_pool(name="t2p", bufs=2))
        t1p = ctx.enter_context(tc.tile_pool(name="t1p", bufs=2))
        rbp = ctx.enter_context(tc.tile_pool(name="rbp", bufs=1))
        otup = ctx.enter_context(tc.tile_pool(name="otup", bufs=2))
        outp = ctx.enter_context(tc.tile_pool(name="outp", bufs=2))
        sip = ctx.enter_context(tc.tile_pool(name="sip", bufs=2))
        # PSUM: ps 2x2 banks + po 1x2 + pss 2x1 = 8 banks.
        pp_s = ctx.enter_context(tc.tile_pool(name="pps", bufs=2, space="PSUM"))
        pp_o = ctx.enter_context(tc.tile_pool(name="ppo", bufs=1, space="PSUM"))
        pp_sum = ctx.enter_context(tc.tile_pool(name="ppsum", bufs=2, space="PSUM"))

        # ---- persistent SBUF ------------------------------------------
        wo_sb = consts.tile([P, DC, HID], F16)
        bias_sb = consts.tile([P, DC], F32)
        ones_sb = consts.tile([P, C], F16)
        nc.vector.memset(ones_sb[:], 1.0)

        q_sb = qkv.tile([P, DC, QL], F16)
        k_sb = qkv.tile([P, DC, L], F16)
        vt_sb = qkv.tile([P, KT, HID], F16)
        ot_sb = qkv.tile([P, DC, QL], F16)

        # ---- DMA loads -------------------------------------------------
        x_sb = ph1.tile([P, DC, L], F16)
        wq_sb = ph1.tile([P, DC, HID], F16)
        wk_sb = ph1.tile([P, DC, HID], F16)
        wv_sb = ph1.tile([P, DC, HID], F16)
        xr = x.rearrange("(a p) n -> p a n", p=P)
        # query-half of x + wq/wk first so the pair-0 Q/K projections and
        # the attention loop start ASAP; wv next for the interleaved V^T.
        nc.scalar.dma_start(out=wq_sb[:], in_=wq.rearrange("(a p) n -> p a n", p=P))
        nc.sync.dma_start(out=x_sb[:, :, 0:N], in_=xr[:, :, 0:N])
        nc.scalar.dma_start(out=x_sb[:, :, N:QL], in_=xr[:, :, N:QL])
        nc.sync.dma_start(out=wk_sb[:], in_=wk.rearrange("(a p) n -> p a n", p=P))
        nc.scalar.dma_start(out=wv_sb[:], in_=wv.rearrange("(a p) n -> p a n", p=P))
        nc.sync.dma_start(out=x_sb[:, :, QL:L], in_=xr[:, :, QL:L])
        nc.scalar.dma_start(out=wo_sb[:], in_=wo.rearrange("(a p) n -> p a n", p=P))
        nc.sync.dma_start(out=bias_sb[:], in_=bias)

        def q_group(m, qc):
            # Q chunk via fp8 DoubleRow: two dc-chunks contracted per pass
            ps = pp_sum.tile([P, N], F32, tag="pss", name=f"qg{m}_{qc}")
            for dc in range(DC):
                nc.tensor.matmul(
                    ps[:],
                    lhsT=wq_sb[:, dc, m * P:(m + 1) * P],
                    rhs=x_sb[:, dc, qc * N:(qc + 1) * N],
                    start=(dc == 0), stop=(dc == DC - 1),
                )
            nc.vector.tensor_copy(q_sb[:, m, qc * N:(qc + 1) * N], ps[:])

        def k_group(m, lc):
            ps = pp_sum.tile([P, N], F32, tag="pss", name=f"kg{m}_{lc}")
            for dc in range(DC):
                nc.tensor.matmul(
                    ps[:],
                    lhsT=wk_sb[:, dc, m * P:(m + 1) * P],
                    rhs=x_sb[:, dc, lc * N:(lc + 1) * N],
                    start=(dc == 0), stop=(dc == DC - 1),
                )
            nc.vector.tensor_copy(k_sb[:, m, lc * N:(lc + 1) * N], ps[:])

        def vt_proj(kt):
            # V^T: [k, hc] (x stationary); interleaved into pair-0's kt loop.
            ps = pp_sum.tile([P, N], F32, tag="pss", name=f"vtp{kt}")
            for dc in range(DC):
                nc.tensor.matmul(
                    ps[:],
                    lhsT=x_sb[:, dc, kt * P:(kt + 1) * P],
                    rhs=wv_sb[:, dc, :],
                    start=(dc == 0), stop=(dc == DC - 1),
                )
            nc.vector.tensor_copy(vt_sb[:, kt, :], ps[:])

        # Deferred projection queue, emitted into the attention loop.
        # Pair 0 (odd kts, 8 slots): K[0] tail + Q[1]/K[1] (needed by the
        # pair-1 scores). Pairs 1-2 (even kts) pick up the rest.
        proj_queue = [("k", 0, 2), ("k", 0, 3), ("q", 1, 0), ("q", 1, 1),
                      ("k", 1, 0), ("k", 1, 1), ("k", 1, 2), ("k", 1, 3)]
        for m in range(2, DC):
            proj_queue.append(("q", m, 0))
            proj_queue.append(("q", m, 1))
            for lc in range(LC):
                proj_queue.append(("k", m, lc))

        def emit_proj_group():
            if not proj_queue:
                return
            kind, m, i = proj_queue.pop(0)
            if kind == "q":
                q_group(m, i)
            else:
                k_group(m, i)

        # ---- prologue: pair-0 Q + K (first half of keys), then most of
        # V^T while the x/w DMAs stream in (PE is otherwise idle here)
        q_group(0, 0)
        k_group(0, 0)
        q_group(0, 1)
        k_group(0, 1)
        for kt0 in range(10):
            vt_proj(kt0)

        def finish_tail(st):
            # denominators part 2: partition-sum via the all-ones [128, 64]
            # stationary matmul, reciprocal, then the deferred
            # normalization multiply (runs early in the NEXT pair).
            m, otu, t1s = st
            rbr = rbp.tile([P, QL], F32, tag="rbr", name=f"rbr{m}")
            for qc in range(QC):
                rb_ps = pp_sum.tile([P, N], F32, tag="pss", name=f"rb{m}_{qc}")
                for half in range(2):
                    nc.tensor.matmul(
                        rb_ps[half * C:(half + 1) * C, :],
                        lhsT=ones_sb[:],
                        rhs=t1s[half][:, qc * N:(qc + 1) * N],
                        start=True, stop=True,
                        tile_position=(0, half * C), skip_group_check=True,
                    )
                nc.vector.reciprocal_approx_fast(
                    out=rbr[:, qc * N:(qc + 1) * N], in_=rb_ps[:]
                )
            nc.vector.tensor_mul(ot_sb[:, m, :], otu[:], rbr[:])

        pending = None
        # ---- attention: one head pair (2m, 2m+1) at a time -------------
        for m in range(DC):
            po = pp_o.tile([P, QL], F32, tag="po", name=f"po{m}")
            t8_A = t8p.tile([P, KT // 2, QL], F16, tag="t8", name=f"t8a{m}")
            t8_B = t8p.tile([P, KT // 2, QL], F16, tag="t8", name=f"t8b{m}")
            t4_A = t4p.tile([P, KT // 4, QL], F16, tag="t4", name=f"t4a{m}")
            t4_B = t4p.tile([P, KT // 4, QL], F16, tag="t4", name=f"t4b{m}")
            last = m == DC - 1
            if last:
                # eager running-fold buffers for a short final chain
                t2_A3 = t2p.tile([P, 2, QL], F16, tag="t2", name="t2a3")
                t2_B3 = t2p.tile([P, 2, QL], F16, tag="t2", name="t2b3")

            def emit_l1(kt, at_prev, at_A, at_B):
                j = kt // 2
                nc.vector.tensor_add(t8_A[:, j, :], at_prev[0][:], at_A[:])
                nc.vector.tensor_add(t8_B[:, j, :], at_prev[1][:], at_B[:])
                if kt % 4 == 3:
                    i = kt // 4
                    nc.vector.tensor_add(
                        t4_A[:, i, :], t8_A[:, 2 * i, :], t8_A[:, 2 * i + 1, :]
                    )
                    nc.vector.tensor_add(
                        t4_B[:, i, :], t8_B[:, 2 * i, :], t8_B[:, 2 * i + 1, :]
                    )
                    if last and i == 1:
                        # t2run = t4[0] + t4[1]
                        nc.vector.tensor_add(
                            t2_A3[:, 0, :], t4_A[:, 0, :], t4_A[:, 1, :]
                        )
                        nc.vector.tensor_add(
                            t2_B3[:, 0, :], t4_B[:, 0, :], t4_B[:, 1, :]
                        )
                    if last and i == 2:
                        # t3run = t2run + t4[2]
                        nc.vector.tensor_add(
                            t2_A3[:, 1, :], t2_A3[:, 0, :], t4_A[:, 2, :]
                        )
                        nc.vector.tensor_add(
                            t2_B3[:, 1, :], t2_B3[:, 0, :], t4_B[:, 2, :]
                        )
                if last and kt == 13:
                    # t3b = t3run + t8[6]  (so the post-loop chain is only
                    # two adds: u = at14+at15; t1 = t3b + u)
                    nc.vector.tensor_add(
                        t2_A3[:, 0, :], t2_A3[:, 1, :], t8_A[:, 6, :]
                    )
                    nc.vector.tensor_add(
                        t2_B3[:, 0, :], t2_B3[:, 1, :], t8_B[:, 6, :]
                    )

            at_prev = [None, None]  # odd-kt pairing for the L1 adds
            for kt in range(KT):
                if m == 0:
                    if kt % 2 == 0 and 2 <= kt <= 12:
                        vt_proj(9 + kt // 2)
                    if kt % 2 == 1:
                        emit_proj_group()
                elif kt % 2 == 0:
                    emit_proj_group()
                if pending is not None and kt == 2:
                    finish_tail(pending)
                    pending = None
                ps_A = pp_s.tile([P, QL], F32, tag="ps")
                ps_B = pp_s.tile([P, QL], F32, tag="ps")
                at_A = atp.tile([P, QL], F16, tag="at")
                at_B = atp.tile([P, QL], F16, tag="at")
                for qc in range(QC):
                    nc.tensor.matmul(
                        ps_A[:, qc * N:(qc + 1) * N],
                        lhsT=k_sb[0:C, m, kt * P:(kt + 1) * P],
                        rhs=q_sb[0:C, m, qc * N:(qc + 1) * N],
                        start=True, stop=True, tile_position=(0, 0),
                    )
                for qc in range(QC):
                    nc.tensor.matmul(
                        ps_B[:, qc * N:(qc + 1) * N],
                        lhsT=k_sb[C:P, m, kt * P:(kt + 1) * P],
                        rhs=q_sb[C:P, m, qc * N:(qc + 1) * N],
                        start=True, stop=True, tile_position=(C, 0),
                    )
                if kt in SEXP_KTS:
                    si = sip.tile([P, QL], I32, tag="si")
                    nc.vector.tensor_scalar(
                        si[:], ps_A[:], SEXP_A, SEXP_B,
                        op0=mybir.AluOpType.mult, op1=mybir.AluOpType.add,
                    )
                    nc.vector.tensor_copy(at_A[:], si[:].bitcast(F32))
                else:
                    nc.scalar.activation(at_A[:], ps_A[:], EXP, scale=SCALE)
                nc.scalar.activation(at_B[:], ps_B[:], EXP, scale=SCALE)
                # attn @ V: col-packed pair, accumulate over kt.
                for qc in range(QC):
                    nc.tensor.matmul(
                        po[0:C, qc * N:(qc + 1) * N],
                        lhsT=vt_sb[:, kt, (2 * m) * C:(2 * m + 1) * C],
                        rhs=at_A[:, qc * N:(qc + 1) * N],
                        start=(kt == 0), stop=(kt == KT - 1),
                        tile_position=(0, 0), skip_group_check=True,
                    )
                    nc.tensor.matmul(
                        po[C:P, qc * N:(qc + 1) * N],
                        lhsT=vt_sb[:, kt, (2 * m + 1) * C:(2 * m + 2) * C],
                        rhs=at_B[:, qc * N:(qc + 1) * N],
                        start=(kt == 0), stop=(kt == KT - 1),
                        tile_position=(0, C), skip_group_check=True,
                    )
                if kt % 2 == 0:
                    at_prev = [at_A, at_B]
                elif kt < KT - 1:
                    emit_l1(kt, at_prev, at_A, at_B)
                else:
                    last_l1 = (kt, at_prev, at_A, at_B)

            otu = otup.tile([P, QL], F16, tag="otu", name=f"otu{m}")
            nc.vector.tensor_copy(otu[:], po[:])

            # denominators part 1: fold to t1 on DVE
            t1s = []
            if last:
                kt, at_prev, at_A, at_B = last_l1
                for at_p, at_k, t2_t in ((at_prev[0], at_A, t2_A3),
                                         (at_prev[1], at_B, t2_B3)):
                    u = t1p.tile([P, QL], F16, tag="t1")
                    nc.vector.tensor_add(u[:], at_p[:], at_k[:])
                    t1 = t1p.tile([P, QL], F16, tag="t1")
                    nc.vector.tensor_add(t1[:], t2_t[:, 0, :], u[:])
                    t1s.append(t1)
            else:
                emit_l1(*last_l1)
                for t4_t in (t4_A, t4_B):
                    r4 = t4_t[:].rearrange("p (a b) q -> p a b q", a=2)
                    t2 = t2p.tile([P, KT // 8, QL], F16, tag="t2")
                    nc.vector.tensor_add(t2[:], r4[:, 0], r4[:, 1])
                    t1 = t1p.tile([P, QL], F16, tag="t1")
                    nc.vector.tensor_add(t1[:], t2[:, 0, :], t2[:, 1, :])
                    t1s.append(t1)
            pending = (m, otu, t1s)

        finish_tail(pending)
        assert not proj_queue, f"{len(proj_queue)} proj groups left"

        # ---- output projection + bias (fp8 DoubleRow) ------------------
        for mo in range(DC):
            for qc in range(QC):
                ps = pp_s.tile([P, QL], F32, tag="ps", name=f"po3_{mo}_{qc}")
                for mh in range(DC):
                    nc.tensor.matmul(
                        ps[:, 0:N],
                        lhsT=wo_sb[:, mh, mo * P:(mo + 1) * P],
                        rhs=ot_sb[:, mh, qc * N:(qc + 1) * N],
                        start=(mh == 0), stop=(mh == DC - 1),
                    )
                ob = outp.tile([P, N], F32, tag="ob")
                nc.vector.tensor_scalar_add(ob[:], ps[:, 0:N], bias_sb[:, mo:mo + 1])
                nc.sync.dma_start(
                    out=out[mo * P:(mo + 1) * P, qc * N:(qc + 1) * N], in_=ob[:]
                )


def build():
    nc = bacc.Bacc("TRN2", target_bir_lowering=False, debug=False)
    x = nc.dram_tensor("x", [D, L], F16, kind="ExternalInput").ap()
    wq = nc.dram_tensor("wq", [D, HID], F16, kind="ExternalInput").ap()
    wk = nc.dram_tensor("wk", [D, HID], F16, kind="ExternalInput").ap()
    wv = nc.dram_tensor("wv", [D, HID], F16, kind="ExternalInput").ap()
    wo = nc.dram_tensor("wo", [HID, D], F16, kind="ExternalInput").ap()
    bias = nc.dram_tensor("bias", [P, DC], F32, kind="ExternalInput").ap()
    out = nc.dram_tensor("out", [D, QL], F32, kind="ExternalOutput").ap()
    with tile.TileContext(nc) as tc:
        emit(nc, tc, x, wq, wk, wv, wo, bias, out)
    nc.compile()
    return nc


_NC_CACHE = None


def _get_nc():
    global _NC_CACHE
    if _NC_CACHE is None:
        _NC_CACHE = build()
    return _NC_CACHE


def make_in_maps(x, w_qkv, w_out, b_out):
    """Host-side sharding: returns the 8 per-core input dicts."""
    from ml_dtypes import bfloat16
    f8 = bfloat16
    # SCALE is applied inside the kernel via the exp's scale parameter
    # (folding it into fp8 wq would land in subnormal range).
    wq_t = np.ascontiguousarray(w_qkv[0:HID].T).astype(f8)
    wk_t = np.ascontiguousarray(w_qkv[HID:2 * HID].T).astype(f8)
    wv_t = np.ascontiguousarray(w_qkv[2 * HID:3 * HID].T).astype(f8)
    wo_t = np.ascontiguousarray(w_out.T).astype(f8)
    bias = np.ascontiguousarray(b_out.reshape(DC, P).T).astype(np.float32)
    in_maps = []
    for core in range(8):
        b, halfq = core // 2, core % 2
        # rotate so this core's query half sits at columns 0:QL; key order
        # is irrelevant (softmax sums over all keys).
        x_rot = np.roll(x[b], -halfq * QL, axis=1).astype(f8)
        in_maps.append({
            "x": np.ascontiguousarray(x_rot),
            "wq": wq_t, "wk": wk_t, "wv": wv_t, "wo": wo_t,
            "bias": bias,
        })
    return in_maps


def assemble(results):
    out = np.zeros((4, D, L), np.float32)
    for core in range(8):
        b, halfq = core // 2, core % 2
        out[b][:, halfq * QL:(halfq + 1) * QL] = results[core]["out"]
    return out


def kernel(x, w_qkv, w_out, b_out):
    x = np.asarray(x, np.float32)
    w_qkv = np.asarray(w_qkv, np.float32)
    w_out = np.asarray(w_out, np.float32)
    b_out = np.asarray(b_out, np.float32)
    nc = _get_nc()
    in_maps = make_in_maps(x, w_qkv, w_out, b_out)
    res = run_bass_kernel_spmd(nc, in_maps, core_ids=list(range(8)))
    return assemble(res.results)


# revision 14
# speedup vs baseline: 1.0313x; 1.0210x over previous
"""Trainium2 Bass kernel for multi-head attention (B=4, H=8, L=2048, dim=512).

Sharding: 8 cores = 4 batches x 2 sequence halves. Each core computes the
full attention output for one batch's 1024-query half (all 8 heads), using
K/V over the full 2048-key sequence; the output projection contracts the
full hidden dim locally, so no cross-core communication is needed.

v3 changes vs baseline:
  - bf16 attention operands (fp16 moving operands stream at ~half rate).
  - The softmax 1/sqrt(d) scale rides the exp's free scale parameter.
  - Prologue DMAs split across the two hardware DGE queues (sync +
    scalar) so the input loads run in parallel.
  - Early-start attention: the prologue projects only pair-0 Q/K plus
    ten V^T tiles (filling the otherwise idle DMA window); the remaining
    projection groups interleave one-per-kt into the attention loop.
  - Shorter end-of-kernel critical chain for the last pair's softmax
    denominators (eager running folds).
"""
import numpy as np

import concourse.bass as bass
import concourse.tile as tile
from concourse import bacc, mybir
from concourse.bass_utils import run_bass_kernel_spmd

F16 = mybir.dt.bfloat16
F32 = mybir.dt.float32
P = 128
D = 512          # model dim
L = 2048         # full sequence (keys)
QL = 1024        # per-core query length
H = 8            # heads
C = 64           # head dim
HID = 512        # H * C
DC = D // P      # 4 contraction chunks
KT = L // P      # 16 key tiles
N = 512          # matmul free-dim chunk
QC = QL // N     # 2 query chunks
LC = L // N      # 4 key free-dim chunks
SCALE = C ** -0.5
EXP = mybir.ActivationFunctionType.Exp
I32 = mybir.dt.int32
# Schraudolph fast-exp: exp(SCALE*s) ~= bitcast_f32(int32(SEXP_A*s + SEXP_B))
SEXP_A = SCALE * (2 ** 23) / float(np.log(2.0))
SEXP_B = float(127 * 2 ** 23 - 490000)
SEXP_KTS = ()  # kts whose A-head exp runs on VectorE instead of ScalarE


def emit(nc, tc, x, wq, wk, wv, wo, bias, out):
    import contextlib
    ctx = contextlib.ExitStack()
    with ctx:
        # ---- pools -----------------------------------------------------
        consts = ctx.enter_context(tc.tile_pool(name="consts", bufs=1))
        qkv = ctx.enter_context(tc.tile_pool(name="qkv", bufs=1))
        ph1 = ctx.enter_context(tc.tile_pool(name="ph1", bufs=1))
        atp = ctx.enter_context(tc.tile_pool(name="atp", bufs=10))
        t8p = ctx.enter_context(tc.tile_pool(name="t8p", bufs=2))
        t4p = ctx.enter_context(tc.tile_pool(name="t4p", bufs=2))
        t2p = ctx.enter_context(tc.tile_pool(name="t2p", bufs=2))
        t1p = ctx.enter_context(tc.tile_pool(name="t1p", bufs=2))
        rbp = ctx.enter_context(tc.tile_pool(name="rbp", bufs=1))
        otup = ctx.enter_context(tc.tile_pool(name="otup", bufs=2))
        outp = ctx.enter_context(tc.tile_pool(name="outp", bufs=2))
        sip = ctx.enter_context(tc.tile_pool(name="sip", bufs=2))
        # PSUM: ps 2x2 banks + po 1x2 + pss 2x1 = 8 banks.
        pp_s = ctx.enter_context(tc.tile_pool(name="pps", bufs=2, space="PSUM"))
        pp_o = ctx.enter_context(tc.tile_pool(name="ppo", bufs=1, space="PSUM"))
        pp_sum = ctx.enter_context(tc.tile_pool(name="ppsum", bufs=2, space="PSUM"))

        # ---- persistent SBUF ------------------------------------------
        wo_sb = consts.tile([P, DC, HID], F16)
        bias_sb = consts.tile([P, DC], F32)
        ones_sb = consts.tile([P, C], F16)
        nc.vector.memset(ones_sb[:], 1.0)

        q_sb = qkv.tile([P, DC, QL], F16)
        k_sb = qkv.tile([P, DC, L], F16)
        vt_sb = qkv.tile([P, KT, HID], F16)
        ot_sb = qkv.tile([P, DC, QL], F16)

        # ---- DMA loads -------------------------------------------------
        x_sb = ph1.tile([P, DC, L], F16)
        wq_sb = ph1.tile([P, DC, HID], F16)
        wk_sb = ph1.tile([P, DC, HID], F16)
        wv_sb = ph1.tile([P, DC, HID], F16)
        xr = x.rearrange("(a p) n -> p a n", p=P)
        # query-half of x + wq/wk first so the pair-0 Q/K projections and
        # the attention loop start ASAP; wv next for the interleaved V^T.
        nc.scalar.dma_start(out=wq_sb[:], in_=wq.rearrange("(a p) n -> p a n", p=P))
        nc.sync.dma_start(out=x_sb[:, :, 0:N], in_=xr[:, :, 0:N])
        nc.scalar.dma_start(out=x_sb[:, :, N:QL], in_=xr[:, :, N:QL])
        nc.sync.dma_start(out=wk_sb[:], in_=wk.rearrange("(a p) n -> p a n", p=P))
        nc.scalar.dma_start(out=wv_sb[:], in_=wv.rearrange("(a p) n -> p a n", p=P))
        nc.sync.dma_start(out=x_sb[:, :, QL:L], in_=xr[:, :, QL:L])
        nc.scalar.dma_start(out=wo_sb[:], in_=wo.rearrange("(a p) n -> p a n", p=P))
        nc.sync.dma_start(out=bias_sb[:], in_=bias)

        def q_group(m, qc):
            # Q chunk via fp8 DoubleRow: two dc-chunks contracted per pass
            ps = pp_sum.tile([P, N], F32, tag="pss", name=f"qg{m}_{qc}")
            for dc in range(DC):
                nc.tensor.matmul(
                    ps[:],
                    lhsT=wq_sb[:, dc, m * P:(m + 1) * P],
                    rhs=x_sb[:, dc, qc * N:(qc + 1) * N],
                    start=(dc == 0), stop=(dc == DC - 1),
                )
            nc.vector.tensor_copy(q_sb[:, m, qc * N:(qc + 1) * N], ps[:])

        def k_group(m, lc):
            ps = pp_sum.tile([P, N], F32, tag="pss", name=f"kg{m}_{lc}")
            for dc in range(DC):
                nc.tensor.matmul(
                    ps[:],
                    lhsT=wk_sb[:, dc, m * P:(m + 1) * P],
                    rhs=x_sb[:, dc, lc * N:(lc + 1) * N],
                    start=(dc == 0), stop=(dc == DC - 1),
                )
            nc.vector.tensor_copy(k_sb[:, m, lc * N:(lc + 1) * N], ps[:])

        def vt_proj(kt):
            # V^T: [k, hc] (x stationary); interleaved into pair-0's kt loop.
            ps = pp_sum.tile([P, N], F32, tag="pss", name=f"vtp{kt}")
            for dc in range(DC):
                nc.tensor.matmul(
                    ps[:],
                    lhsT=x_sb[:, dc, kt * P:(kt + 1) * P],
                    rhs=wv_sb[:, dc, :],
                    start=(dc == 0), stop=(dc == DC - 1),
                )
            nc.vector.tensor_copy(vt_sb[:, kt, :], ps[:])

        # Deferred projection queue, emitted into the attention loop.
        # Pair 0 (odd kts, 8 slots): K[0] tail + Q[1]/K[1] (needed by the
        # pair-1 scores). Pairs 1-2 (even kts) pick up the rest.
        proj_queue = [("k", 0, 2), ("k", 0, 3), ("q", 1, 0), ("q", 1, 1),
                      ("k", 1, 0), ("k", 1, 1), ("k", 1, 2), ("k", 1, 3)]
        for m in range(2, DC):
            proj_queue.append(("q", m, 0))
            proj_queue.append(("q", m, 1))
            for lc in range(LC):
                proj_queue.append(("k", m, lc))

        def emit_proj_group():
            if not proj_queue:
                return
            kind, m, i = proj_queue.pop(0)
            if kind == "q":
                q_group(m, i)
            else:
                k_group(m, i)

        # ---- prologue: pair-0 Q + K (first half of keys), then most of
        # V^T while the x/w DMAs stream in (PE is otherwise idle here)
        q_group(0, 0)
        k_group(0, 0)
        q_group(0, 1)
        k_group(0, 1)
        for kt0 in range(10):
            vt_proj(kt0)

        def finish_tail(st):
            # denominators part 2: partition-sum via the all-ones [128, 64]
            # stationary matmul, reciprocal, then the deferred
            # normalization multiply (runs early in the NEXT pair).
            m, otu, t1s = st
            rbr = rbp.tile([P, QL], F32, tag="rbr", name=f"rbr{m}")
            for qc in range(QC):
                rb_ps = pp_sum.tile([P, N], F32, tag="pss", name=f"rb{m}_{qc}")
                for half in range(2):
                    nc.tensor.matmul(
                        rb_ps[half * C:(half + 1) * C, :],
                        lhsT=ones_sb[:],
                        rhs=t1s[half][:, qc * N:(qc + 1) * N],
                        start=True, stop=True,
                        tile_position=(0, half * C), skip_group_check=True,
                    )
                nc.vector.reciprocal_approx_fast(
                    out=rbr[:, qc * N:(qc + 1) * N], in_=rb_ps[:]
                )
            nc.vector.tensor_mul(ot_sb[:, m, :], otu[:], rbr[:])

        pending = None
        # ---- attention: one head pair (2m, 2m+1) at a time -------------
        for m in range(DC):
            po = pp_o.tile([P, QL], F32, tag="po", name=f"po{m}")
            t8_A = t8p.tile([P, KT // 2, QL], F16, tag="t8", name=f"t8a{m}")
            t8_B = t8p.tile([P, KT // 2, QL], F16, tag="t8", name=f"t8b{m}")
            t4_A = t4p.tile([P, KT // 4, QL], F16, tag="t4", name=f"t4a{m}")
            t4_B = t4p.tile([P, KT // 4, QL], F16, tag="t4", name=f"t4b{m}")
            last = m == DC - 1
            if last:
                # eager running-fold buffers for a short final chain
                t2_A3 = t2p.tile([P, 2, QL], F16, tag="t2", name="t2a3")
                t2_B3 = t2p.tile([P, 2, QL], F16, tag="t2", name="t2b3")

            def emit_l1(kt, at_prev, at_A, at_B):
                j = kt // 2
                nc.vector.tensor_add(t8_A[:, j, :], at_prev[0][:], at_A[:])
                nc.vector.tensor_add(t8_B[:, j, :], at_prev[1][:], at_B[:])
                if kt % 4 == 3:
                    i = kt // 4
                    nc.vector.tensor_add(
                        t4_A[:, i, :], t8_A[:, 2 * i, :], t8_A[:, 2 * i + 1, :]
                    )
                    nc.vector.tensor_add(
                        t4_B[:, i, :], t8_B[:, 2 * i, :], t8_B[:, 2 * i + 1, :]
                    )
                    if last and i == 1:
                        # t2run = t4[0] + t4[1]
                        nc.vector.tensor_add(
                            t2_A3[:, 0, :], t4_A[:, 0, :], t4_A[:, 1, :]
                        )
                        nc.vector.tensor_add(
                            t2_B3[:, 0, :], t4_B[:, 0, :], t4_B[:, 1, :]
                        )
                    if last and i == 2:
                        # t3run = t2run + t4[2]
                        nc.vector.tensor_add(
                            t2_A3[:, 1, :], t2_A3[:, 0, :], t4_A[:, 2, :]
                        )
                        nc.vector.tensor_add(
                            t2_B3[:, 1, :], t2_B3[:, 0, :], t4_B[:, 2, :]
                        )
                if last and kt == 13:
                    # t3b = t3run + t8[6]  (so the post-loop chain is only
                    # two adds: u = at14+at15; t1 = t3b + u)
                    nc.vector.tensor_add(
                        t2_A3[:, 0, :], t2_A3[:, 1, :], t8_A[:, 6, :]
                    )
                    nc.vector.tensor_add(
                        t2_B3[:, 0, :], t2_B3[:, 1, :], t8_B[:, 6, :]
                    )

            at_prev = [None, None]  # odd-kt pairing for the L1 adds
            for kt in range(KT):
                if m == 0:
                    if kt % 2 == 0 and 2 <= kt <= 12:
                        vt_proj(9 + kt // 2)
                    if kt % 2 == 1:
                        emit_proj_group()
                elif kt % 2 == 0:
                    emit_proj_group()
                ps_A = pp_s.tile([P, QL], F32, tag="ps")
                ps_B = pp_s.tile([P, QL], F32, tag="ps")
                at_A = atp.tile([P, QL], F16, tag="at")
                at_B = atp.tile([P, QL], F16, tag="at")
                for qc in range(QC):
                    nc.tensor.matmul(
                        ps_A[:, qc * N:(qc + 1) * N],
                        lhsT=k_sb[0:C, m, kt * P:(kt + 1) * P],
                        rhs=q_sb[0:C, m, qc * N:(qc + 1) * N],
                        start=True, stop=True, tile_position=(0, 0),
                    )
                for qc in range(QC):
                    nc.tensor.matmul(
                        ps_B[:, qc * N:(qc + 1) * N],
                        lhsT=k_sb[C:P, m, kt * P:(kt + 1) * P],
                        rhs=q_sb[C:P, m, qc * N:(qc + 1) * N],
                        start=True, stop=True, tile_position=(C, 0),
                    )
                if kt in SEXP_KTS:
                    si = sip.tile([P, QL], I32, tag="si")
                    nc.vector.tensor_scalar(
                        si[:], ps_A[:], SEXP_A, SEXP_B,
                        op0=mybir.AluOpType.mult, op1=mybir.AluOpType.add,
                    )
                    nc.vector.tensor_copy(at_A[:], si[:].bitcast(F32))
                else:
                    nc.scalar.activation(at_A[:], ps_A[:], EXP, scale=SCALE)
                nc.scalar.activation(at_B[:], ps_B[:], EXP, scale=SCALE)
                # attn @ V: col-packed pair, accumulate over kt.
                for qc in range(QC):
                    nc.tensor.matmul(
                        po[0:C, qc * N:(qc + 1) * N],
                        lhsT=vt_sb[:, kt, (2 * m) * C:(2 * m + 1) * C],
                        rhs=at_A[:, qc * N:(qc + 1) * N],
                        start=(kt == 0), stop=(kt == KT - 1),
                        tile_position=(0, 0), skip_group_check=True,
                    )
                    nc.tensor.matmul(
                        po[C:P, qc * N:(qc + 1) * N],
                        lhsT=vt_sb[:, kt, (2 * m + 1) * C:(2 * m + 2) * C],
                        rhs=at_B[:, qc * N:(qc + 1) * N],
                        start=(kt == 0), stop=(kt == KT - 1),
                        tile_position=(0, C), skip_group_check=True,
                    )
                if pending is not None and kt == 4:
                    finish_tail(pending)
                    pending = None
                if kt % 2 == 0:
                    at_prev = [at_A, at_B]
                elif kt < KT - 1:
                    emit_l1(kt, at_prev, at_A, at_B)
                else:
                    last_l1 = (kt, at_prev, at_A, at_B)

            otu = otup.tile([P, QL], F16, tag="otu", name=f"otu{m}")
            nc.vector.tensor_copy(otu[:], po[:])

            # denominators part 1: fold to t1 on DVE
            t1s = []
            if last:
                kt, at_prev, at_A, at_B = last_l1
                for at_p, at_k, t2_t in ((at_prev[0], at_A, t2_A3),
                                         (at_prev[1], at_B, t2_B3)):
                    u = t1p.tile([P, QL], F16, tag="t1")
                    nc.vector.tensor_add(u[:], at_p[:], at_k[:])
                    t1 = t1p.tile([P, QL], F16, tag="t1")
                    nc.vector.tensor_add(t1[:], t2_t[:, 0, :], u[:])
                    t1s.append(t1)
            else:
                emit_l1(*last_l1)
                for t4_t in (t4_A, t4_B):
                    r4 = t4_t[:].rearrange("p (a b) q -> p a b q", a=2)
                    t2 = t2p.tile([P, KT // 8, QL], F16, tag="t2")
                    nc.vector.tensor_add(t2[:], r4[:, 0], r4[:, 1])
                    t1 = t1p.tile([P, QL], F16, tag="t1")
                    nc.vector.tensor_add(t1[:], t2[:, 0, :], t2[:, 1, :])
                    t1s.append(t1)
            pending = (m, otu, t1s)

        finish_tail(pending)
        assert not proj_queue, f"{len(proj_queue)} proj groups left"

        # ---- output projection + bias (fp8 DoubleRow) ------------------
        for mo in range(DC):
            for qc in range(QC):
                ps = pp_s.tile([P, QL], F32, tag="ps", name=f"po3_{mo}_{qc}")
                for mh in range(DC):
                    nc.tensor.matmul(
                        ps[:, 0:N],
                        lhsT=wo_sb[:, mh, mo * P:(mo + 1) * P],
                        rhs=ot_sb[:, mh, qc * N:(qc + 1) * N],
                        start=(mh == 0), stop=(mh == DC - 1),
                    )
                ob = outp.tile([P, N], F32, tag="ob")
                nc.vector.tensor_scalar_add(ob[:], ps[:, 0:N], bias_sb[:, mo:mo + 1])
                nc.sync.dma_start(
                    out=out[mo * P:(mo + 1) * P, qc * N:(qc + 1) * N], in_=ob[:]
                )


def build():
    nc = bacc.Bacc("TRN2", target_bir_lowering=False, debug=False)
    x = nc.dram_tensor("x", [D, L], F16, kind="ExternalInput").ap()
    wq = nc.dram_tensor("wq", [D, HID], F16, kind="ExternalInput").ap()
    wk = nc.dram_tensor("wk", [D, HID], F16, kind="ExternalInput").ap()
    wv = nc.dram_tensor("wv", [D, HID], F16, kind="ExternalInput").ap()
    wo = nc.dram_tensor("wo", [HID, D], F16, kind="ExternalInput").ap()
    bias = nc.dram_tensor("bias", [P, DC], F32, kind="ExternalInput").ap()
    out = nc.dram_tensor("out", [D, QL], F32, kind="ExternalOutput").ap()
    with tile.TileContext(nc) as tc:
        emit(nc, tc, x, wq, wk, wv, wo, bias, out)
    nc.compile()
    return nc


_NC_CACHE = None


def _get_nc():
    global _NC_CACHE
    if _NC_CACHE is None:
        _NC_CACHE = build()
    return _NC_CACHE


def make_in_maps(x, w_qkv, w_out, b_out):
    """Host-side sharding: returns the 8 per-core input dicts."""
    from ml_dtypes import bfloat16
    f8 = bfloat16
    # SCALE is applied inside the kernel via the exp's scale parameter
    # (folding it into fp8 wq would land in subnormal range).
    wq_t = np.ascontiguousarray(w_qkv[0:HID].T).astype(f8)
    wk_t = np.ascontiguousarray(w_qkv[HID:2 * HID].T).astype(f8)
    wv_t = np.ascontiguousarray(w_qkv[2 * HID:3 * HID].T).astype(f8)
    wo_t = np.ascontiguousarray(w_out.T).astype(f8)
    bias = np.ascontiguousarray(b_out.reshape(DC, P).T).astype(np.float32)
    in_maps = []
    for core in range(8):
        b, halfq = core // 2, core % 2
        # rotate so this core's query half sits at columns 0:QL; key order
        # is irrelevant (softmax sums over all keys).
        x_rot = np.roll(x[b], -halfq * QL, axis=1).astype(f8)
        in_maps.append({
            "x": np.ascontiguousarray(x_rot),
            "wq": wq_t, "wk": wk_t, "wv": wv_t, "wo": wo_t,
            "bias": bias,
        })
    return in_maps


def assemble(results):
    out = np.zeros((4, D, L), np.float32)
    for core in range(8):
        b, halfq = core // 2, core % 2
        out[b][:, halfq * QL:(halfq + 1) * QL] = results[core]["out"]
    return out


def kernel(x, w_qkv, w_out, b_out):
    x = np.asarray(x, np.float32)
    w_qkv = np.asarray(w_qkv, np.float32)
    w_out = np.asarray(w_out, np.float32)
    b_out = np.asarray(b_out, np.float32)
    nc = _get_nc()
    in_maps = make_in_maps(x, w_qkv, w_out, b_out)
    res = run_bass_kernel_spmd(nc, in_maps, core_ids=list(range(8)))
    return assemble(res.results)
